# revision 16
# baseline (speedup 1.0000x reference)
"""Trainium2 Bass kernel for nn_CrossAttention (B=8, L=K=512, M=N=P=D=64).

One batch per NeuronCore (8 cores, data-parallel, no collectives).

Math per batch b:
  scoresT[k,l] = scale * (K @ Q^T)                  # PE, contract D=64
  ET = exp(scale*scoresT)                           # ACT (no max-sub: |s|<~45)
  VE[k,p,n] = vexp[k,p] broadcast over n            # ACT copy (cast bf16)
  vkc[k,n] = sum_p vk[k,p,n]*VE[k,p,n]              # DVE bf16 2x mult + tree
  [tmp|sums][l,(n|1)] = ET_chunk^T @ [vkc|1]        # PE accumulate, l on partitions
  tmpn[l,n] = tmp[l,n]/sums[l]                      # ACT copy with per-partition scale
  attn[l,m] = sum_n vq[l,m,n]*tmpn[l,n]             # DVE bf16 2x mult + tree
  out = LN(attn + q)*gamma + beta                   # DVE stats, ACT sqrt

Layout: all row-indexed tensors use a (p,t) interleave — partition p of
tile t holds logical row 4p+t — so every DMA moves >=1KB contiguous per
partition (line rate). The interleave is self-consistent along both k
and l and the single output store undoes it.

Schedule (from NTFF traces of previous versions):
  - per-core HBM caps ~430 GB/s TOTAL across queues, so all 16MB of bulk
    (vk then vq, f32->bf16 cast in flight) rides ONE SWDGE queue whose
    FIFO gives vk strict priority; q/k/vexp ride the sync queue (done
    ~1.5us); only the slow to_broadcast splats (gamma/beta) and the
    final store use the scalar queue.
  - gpsimd runs NO compute: DVE ops overlapping gpsimd tensor ops were
    measured 4-8x slower (SBUF contention).
  - the A-step product runs at DVE 2x by materializing the vexp
    broadcast into a real bf16 tile on the otherwise-idle ACT engine
    (a stride-0 innermost operand would force 1x on the DVE).
  - scale is broadcast on-chip via a ones-row PE matmul instead of a
    128-descriptor splat DMA (it gates the exps).
"""

import numpy as np

B = 8
L = 512
KK = 512
MM = 64
NN = 64
PP = 64
DD = 64
NCORES = 8

LT = L // 128   # 4 l-tiles (slot index in the (p,t) interleave)
KT = KK // 128  # 4 k-tiles
HALF = 32       # p-half / m-half within a tile

_CACHE = {}


def _patch_multiwait_split():
    """This environment's walrus accepts only ONE sem-wait per instruction,
    while Tile emits instructions carrying several. Rewrite the BIR JSON just
    before compilation: hoist excess waits onto single-wait NoOps inserted
    immediately before the offending instruction on the same engine."""
    import json

    from concourse import bass_utils, bass2jax

    if getattr(bass_utils, "_multiwait_split_patched", False):
        return

    orig = bass_utils.compile_bir_kernel

    def _split(bir_json):
        if isinstance(bir_json, bytes):
            m = json.loads(bir_json.decode())
        else:
            m = json.loads(bir_json)
        cnt = 0
        for fn in m["functions"]:
            for bb in fn["blocks"]:
                insts = bb["instructions"]
                out = []
                for inst in insts:
                    si = inst.get("sync_info")
                    waits = si.get("on_wait", []) if si else []
                    if len(waits) > 1:
                        for w in waits[:-1]:
                            cnt += 1
                            out.append(
                                {
                                    "name": f"WS-{cnt}-{inst['name']}",
                                    "opcode": "NoOp",
                                    "engine": inst["engine"],
                                    "ins": [],
                                    "outs": [],
                                    "debug": inst.get("debug", 0),
                                    "sync_info": {
                                        "on_update": [],
                                        "on_wait": [w],
                                    },
                                }
                            )
                        si["on_wait"] = [waits[-1]]
                    out.append(inst)
                bb["instructions"] = out
        return json.dumps(m).encode()

    def patched(bir_json, tmpdir, neff_name="file.neff", **kw):
        return orig(_split(bir_json), tmpdir, neff_name=neff_name, **kw)

    bass_utils.compile_bir_kernel = patched
    bass2jax.compile_bir_kernel = patched
    bass_utils._multiwait_split_patched = True


def _build_nc():
    import contextlib

    import concourse.bass as bass
    import concourse.tile as tile
    from concourse import mybir
    from concourse.masks import make_identity

    _patch_multiwait_split()

    f32 = mybir.dt.float32
    bf16 = mybir.dt.bfloat16
    Alu = mybir.AluOpType
    Act = mybir.ActivationFunctionType

    nc = bass.Bass()
    q_d = nc.dram_tensor("q", [L, DD], f32, kind="ExternalInput")
    k_d = nc.dram_tensor("k", [KK, DD], f32, kind="ExternalInput")
    vq_d = nc.dram_tensor("vq", [L, MM * NN], f32, kind="ExternalInput")
    vk_d = nc.dram_tensor("vk", [KK, PP * NN], f32, kind="ExternalInput")
    vexp_d = nc.dram_tensor("vexp", [KK, PP], f32, kind="ExternalInput")
    scale_d = nc.dram_tensor("scale", [1, 1], f32, kind="ExternalInput")
    gamma_d = nc.dram_tensor("ln_gamma", [1, DD], f32, kind="ExternalInput")
    beta_d = nc.dram_tensor("ln_beta", [1, DD], f32, kind="ExternalInput")
    out_d = nc.dram_tensor("out", [L, MM], f32, kind="ExternalOutput")

    # (p,t)-interleaved DRAM views: partition p, slot t -> logical row 4p+t
    vk_v = vk_d[:].rearrange("(p t) c -> p t c", t=KT)
    vq_v = vq_d[:].rearrange("(p t) c -> p t c", t=LT)

    with tile.TileContext(nc) as tc:
        lp_cm = nc.allow_low_precision("bf16 value-path partial sums")
        with lp_cm, contextlib.ExitStack() as ctx:
            const = ctx.enter_context(tc.tile_pool(name="const", bufs=1))
            vk_pool = ctx.enter_context(tc.tile_pool(name="vk", bufs=2 * KT))
            vq_pool = ctx.enter_context(tc.tile_pool(name="vq", bufs=LT))
            ve_pool = ctx.enter_context(tc.tile_pool(name="ve", bufs=3))
            prod_pool = ctx.enter_context(tc.tile_pool(name="prod", bufs=3))
            tree_pool = ctx.enter_context(tc.tile_pool(name="tree", bufs=2))
            small = ctx.enter_context(tc.tile_pool(name="small", bufs=2))
            ps_scores = ctx.enter_context(
                tc.tile_pool(name="ps_s", bufs=2, space="PSUM")
            )
            ps_tmp_pool = ctx.enter_context(
                tc.tile_pool(name="ps_tmp", bufs=1, space="PSUM")
            )
            ps_tr = ctx.enter_context(tc.tile_pool(name="ps_tr", bufs=1, space="PSUM"))

            # ---- small line-rate loads on the sync HWDGE queue ----
            scale_sb = const.tile([1, 1], f32)
            nc.sync.dma_start(out=scale_sb, in_=scale_d[:])
            q_nat = const.tile([128, LT, DD], f32)
            nc.sync.dma_start(
                out=q_nat[:].rearrange("p a b -> p (a b)"),
                in_=q_d[:].rearrange("(p t) d -> p (t d)", t=LT),
            )
            k_nat = const.tile([128, KT, DD], f32)
            nc.sync.dma_start(
                out=k_nat[:].rearrange("p a b -> p (a b)"),
                in_=k_d[:].rearrange("(p t) d -> p (t d)", t=KT),
            )
            # slow splat DMAs isolated on the scalar queue (needed late)
            gamma_bc = const.tile([128, DD], f32)
            nc.scalar.dma_start(out=gamma_bc, in_=gamma_d[:].to_broadcast([128, DD]))
            beta_bc = const.tile([128, DD], f32)
            nc.scalar.dma_start(out=beta_bc, in_=beta_d[:].to_broadcast([128, DD]))

            # ---- bulk: ONE SWDGE queue, bf16 cast in flight ----
            # vexp leads (it gates the VE copies and so the whole A phase)
            vexp_nat = const.tile([128, KT, PP], f32)
            nc.gpsimd.dma_start(
                out=vexp_nat[:].rearrange("p a b -> p (a b)"),
                in_=vexp_d[:].rearrange("(p t) d -> p (t d)", t=KT),
            )
            vk_halves = []
            for h in range(KT * 2):
                i, hh = divmod(h, 2)
                vkh = vk_pool.tile([128, HALF, NN], bf16, tag="vk", name=f"vkh{h}")
                nc.gpsimd.dma_start(
                    out=vkh,
                    in_=vk_v[:, i, hh * HALF * NN : (hh + 1) * HALF * NN],
                )
                vk_halves.append(vkh)
            vq_tiles = []
            for j in range(LT):
                vqt = vq_pool.tile([128, MM, NN], bf16, tag="vq", name=f"vqt{j}")
                nc.gpsimd.dma_start(out=vqt, in_=vq_v[:, j, :])
                vq_tiles.append(vqt)

            # ---- front-end chain, force-scheduled early: identity,
            # scale broadcast, tile-0 VE copies, transposes, scores, exps.
            # Without this the compile-time scheduler lets the 16us of VE
            # copies clog ACT and the exps (and so the tmp matmuls and the
            # whole C phase) slip by ~25us.
            exp_warm = const.tile([128, 1], f32)
            identity = const.tile([128, 128], f32)
            zero_t = const.tile([128, 1], f32)
            eps_t = const.tile([128, 1], f32)
            ones_row = const.tile([1, 128], f32)
            vkc = const.tile([128, KT, NN + 1], f32)
            qT = const.tile([64, L], f32)
            kT = const.tile([64, KK], f32)
            ET = const.tile([128, KT, L], f32)
            scale_bc = const.tile([128, 1], f32)
            ve_tiles = {}
            with tc.high_priority():
                make_identity(nc, identity)
                nc.vector.memset(zero_t, 0.0)
                nc.vector.memset(eps_t, 1e-3)
                nc.vector.memset(ones_row, 1.0)
                # vkc carries a fused ones column (65th) so one matmul per
                # (k-tile, l-chunk) produces both tmp and the denominator
                nc.vector.memset(vkc[:, :, NN : NN + 1], 1.0)

                # trigger the Exp act-table load immediately so the first
                # real exp does not eat the 1.28us table latency
                nc.scalar.activation(
                    exp_warm, eps_t, func=Act.Exp, bias=0.0, scale=1.0
                )
                ps_bc = ps_tr.tile([128, 1], f32, tag="bc")
                nc.tensor.matmul(
                    ps_bc, lhsT=ones_row, rhs=scale_sb, start=True, stop=True
                )
                nc.scalar.copy(scale_bc, ps_bc)

                # tile-0 VE copies first so the A phase can start while the
                # exps are still waiting on the scores
                for hh in range(2):
                    ve0 = ve_pool.tile(
                        [128, HALF, NN], bf16, tag="ve", name=f"ve0{hh}"
                    )
                    nc.scalar.activation(
                        ve0[:],
                        vexp_nat[
                            :, 0, hh * HALF : (hh + 1) * HALF, None
                        ].to_broadcast([128, HALF, NN]),
                        func=Act.Copy,
                        bias=0.0,
                        scale=1.0,
                    )
                    ve_tiles[(0, hh)] = ve0


            # preload the Sqrt act table now (off the critical path); the
            # LN tail would otherwise eat a 1.3us table swap
            dummy = const.tile([128, 1], f32)
            nc.scalar.activation(dummy, eps_t, func=Act.Sqrt, bias=0.0, scale=1.0)

            # ---- A step + tmp/sums matmuls per k-tile ----
            ps_tmps = [
                ps_tmp_pool.tile(
                    [128, NN + 1], f32, tag=f"tmp{j}", name=f"ps_tmp{j}"
                )
                for j in range(LT)
            ]

            def dve_tree_p(pr, out_slot, w0):
                # reduce [128, w0, NN] over the middle axis down to out_slot
                cur = pr
                w = w0 // 2
                while w >= 1:
                    if w == 1:
                        nxt = out_slot
                    else:
                        nxt = tree_pool.tile(
                            [128, w, NN], bf16, tag=f"at{w}", name=f"at{w}"
                        )
                    nc.vector.tensor_tensor(
                        nxt[:], cur[:, 0:w, :], cur[:, w : 2 * w, :], Alu.add
                    )
                    cur = nxt
                    w //= 2

            def emit_transposes(which):
                # demoted priority: the in-order DVE stream must not block
                # the A products on the q/k loads via these PSUM copies
                src_t, dst_t = (q_nat, qT) if which == "q" else (k_nat, kT)
                n_t = LT if which == "q" else KT
                with tc.high_priority(offset=-60):
                    for i in range(n_t):
                        pt = ps_tr.tile([64, 128], f32, tag="tr")
                        nc.tensor.transpose(pt, src_t[:, i, :], identity)
                        nc.vector.tensor_copy(
                            dst_t[:, i * 128 : (i + 1) * 128], pt
                        )

            def emit_scores():
                for i in range(KT):
                    ps_s = ps_scores.tile([128, L], f32, tag="sc")
                    nc.tensor.matmul(
                        ps_s,
                        lhsT=kT[:, i * 128 : (i + 1) * 128],
                        rhs=qT[:],
                        start=True,
                        stop=True,
                    )
                    nc.scalar.activation(
                        ET[:, i, :],
                        ps_s,
                        func=Act.Exp,
                        bias=zero_t[:],
                        scale=scale_bc[:],
                    )

            for i in range(KT):
                pr = prod_pool.tile([128, PP, NN], bf16, tag="pr", name=f"pr{i}")
                for hh in range(2):
                    # materialize the vexp broadcast as a real bf16 tile on
                    # ACT so the DVE product gets 2x (packed operands only)
                    ve = ve_tiles.get((i, hh))
                    if ve is None:
                        ve = ve_pool.tile(
                            [128, HALF, NN], bf16, tag="ve", name=f"ve{i}{hh}"
                        )
                        nc.scalar.activation(
                            ve[:],
                            vexp_nat[
                                :, i, hh * HALF : (hh + 1) * HALF, None
                            ].to_broadcast([128, HALF, NN]),
                            func=Act.Copy,
                            bias=0.0,
                            scale=1.0,
                        )
                    nc.vector.tensor_tensor(
                        pr[:, hh * HALF : (hh + 1) * HALF, :],
                        vk_halves[2 * i + hh][:],
                        ve[:],
                        Alu.mult,
                    )
                    if i == 0 and hh == 0:
                        emit_transposes("q")
                    elif i == 0 and hh == 1:
                        emit_transposes("k")
                        emit_scores()
                dve_tree_p(pr, vkc[:, i : i + 1, 0:NN], PP)
                for j in range(LT):
                    nc.tensor.matmul(
                        ps_tmps[j],
                        lhsT=ET[:, i, j * 128 : (j + 1) * 128],
                        rhs=vkc[:, i, :],
                        start=(i == 0),
                        stop=(i == KT - 1),
                    )

            # ---- fold 1/sums into tmp during the PSUM->SBUF copy ----
            tmp_sb = const.tile([128, LT, NN], bf16)
            for j in range(LT):
                recip = small.tile([128, 1], f32, tag="recip")
                nc.vector.reciprocal(recip, ps_tmps[j][:, NN : NN + 1])
                nc.scalar.activation(
                    tmp_sb[:, j, :],
                    ps_tmps[j][:, 0:NN],
                    func=Act.Copy,
                    bias=0.0,
                    scale=recip[:],
                )

            # ---- C step per l-tile: product+tree per m-half, then LN ----
            out_sb = const.tile([128, LT, MM], f32)
            for j in range(LT):
                attn = small.tile([128, MM], f32, tag="attn")
                pr2 = prod_pool.tile([128, MM, NN], bf16, tag="pr2", name=f"pr2{j}")
                nc.vector.tensor_tensor(
                    pr2[:],
                    vq_tiles[j][:],
                    tmp_sb[:, j, None, :].to_broadcast([128, MM, NN]),
                    Alu.mult,
                )
                cur = pr2
                w = NN // 2
                while w >= 1:
                    if w == 1:
                        nxt = attn[:, :, None]
                    else:
                        nxt = tree_pool.tile(
                            [128, MM, w], bf16, tag=f"ct{w}", name=f"ct{w}"
                        )
                    nc.vector.tensor_tensor(
                        nxt[:], cur[:, :, 0:w], cur[:, :, w : 2 * w], Alu.add
                    )
                    cur = nxt
                    w //= 2

                x = small.tile([128, MM], f32, tag="x")
                nc.vector.tensor_tensor(x, attn, q_nat[:, j, :], Alu.add)
                stats = small.tile([128, 6], f32, tag="stats")
                nc.vector.bn_stats(out=stats, in_=x[:])
                mv = small.tile([128, 2], f32, tag="mv")
                nc.vector.bn_aggr(out=mv, in_=stats[:])
                sd = small.tile([128, 1], f32, tag="sd")
                nc.scalar.activation(
                    sd, mv[:, 1:2], func=Act.Sqrt, bias=eps_t[:], scale=1.0
                )
                rstd = small.tile([128, 1], f32, tag="rstd")
                nc.vector.reciprocal(rstd, sd)
                xn = small.tile([128, MM], f32, tag="xn")
                nc.vector.tensor_scalar(
                    out=xn, in0=x, scalar1=mv[:, 0:1], scalar2=rstd,
                    op0=Alu.subtract, op1=Alu.mult,
                )
                xg = small.tile([128, MM], f32, tag="xg")
                nc.vector.tensor_tensor(xg, xn, gamma_bc, Alu.mult)
                nc.vector.tensor_tensor(out_sb[:, j, :], xg, beta_bc, Alu.add)

            # single line-rate store of the whole output
            nc.scalar.dma_start(
                out=out_d[:].rearrange("(p t) d -> p t d", t=LT), in_=out_sb
            )

    return nc


def _get_nc():
    if "nc" not in _CACHE:
        _CACHE["nc"] = _build_nc()
    return _CACHE["nc"]


def kernel(q, k, vq, vk, vexp, scale, ln_gamma, ln_beta):
    from concourse import bass_utils

    nc = _get_nc()
    q = np.ascontiguousarray(np.asarray(q, dtype=np.float32))
    k = np.ascontiguousarray(np.asarray(k, dtype=np.float32))
    vq = np.ascontiguousarray(np.asarray(vq, dtype=np.float32)).reshape(B, L, MM * NN)
    vk = np.ascontiguousarray(np.asarray(vk, dtype=np.float32)).reshape(B, KK, PP * NN)
    vexp = np.ascontiguousarray(np.asarray(vexp, dtype=np.float32))
    scale_arr = np.asarray(scale, dtype=np.float32).reshape(1, 1)
    gamma_arr = np.asarray(ln_gamma, dtype=np.float32).reshape(1, DD)
    beta_arr = np.asarray(ln_beta, dtype=np.float32).reshape(1, DD)

    in_maps = [
        {
            "q": q[c],
            "k": k[c],
            "vq": vq[c],
            "vk": vk[c],
            "vexp": vexp[c],
            "scale": scale_arr,
            "ln_gamma": gamma_arr,
            "ln_beta": beta_arr,
        }
        for c in range(NCORES)
    ]
    res = bass_utils.run_bass_kernel_spmd(nc, in_maps, core_ids=list(range(NCORES)))
    out = np.stack([res.results[c]["out"] for c in range(NCORES)], axis=0)
    return out.astype(np.float32)


# revision 17
# speedup vs baseline: 1.1391x; 1.1391x over previous
"""Trainium2 Bass kernel for nn_CrossAttention (B=8, L=K=512, M=N=P=D=64).

One batch per NeuronCore (8 cores, data-parallel, no collectives).

Math per batch b:
  scoresT[k,l] = scale * (K @ Q^T)                  # PE, contract D=64
  ET = exp(scale*scoresT)                           # ACT (no max-sub: |s|<~45)
  VE[k,p,n] = vexp[k,p] broadcast over n            # ACT copy (cast bf16)
  vkc[k,n] = sum_p vk[k,p,n]*VE[k,p,n]              # DVE bf16 2x mult + tree
  [tmp|sums][l,(n|1)] = ET_chunk^T @ [vkc|1]        # PE accumulate, l on partitions
  tmpn[l,n] = tmp[l,n]/sums[l]                      # ACT copy with per-partition scale
  attn[l,m] = sum_n vq[l,m,n]*tmpn[l,n]             # DVE bf16 2x mult + tree
  out = LN(attn + q)*gamma + beta                   # DVE stats, ACT sqrt

Layout: all row-indexed tensors use a (p,t) interleave — partition p of
tile t holds logical row 4p+t — so every DMA moves >=1KB contiguous per
partition (line rate). The interleave is self-consistent along both k
and l and the single output store undoes it.

Schedule (from NTFF traces of previous versions):
  - per-core HBM caps ~430 GB/s TOTAL across queues, so all 16MB of bulk
    (vk then vq, f32->bf16 cast in flight) rides ONE SWDGE queue whose
    FIFO gives vk strict priority; q/k/vexp ride the sync queue (done
    ~1.5us); only the slow to_broadcast splats (gamma/beta) and the
    final store use the scalar queue.
  - gpsimd runs NO compute: DVE ops overlapping gpsimd tensor ops were
    measured 4-8x slower (SBUF contention).
  - the A-step product runs at DVE 2x by materializing the vexp
    broadcast into a real bf16 tile on the otherwise-idle ACT engine
    (a stride-0 innermost operand would force 1x on the DVE).
  - scale is broadcast on-chip via a ones-row PE matmul instead of a
    128-descriptor splat DMA (it gates the exps).
"""

import numpy as np

B = 8
L = 512
KK = 512
MM = 64
NN = 64
PP = 64
DD = 64
NCORES = 8

LT = L // 128   # 4 l-tiles (slot index in the (p,t) interleave)
KT = KK // 128  # 4 k-tiles
HALF = 32       # p-half / m-half within a tile

_CACHE = {}


def _patch_multiwait_split():
    """This environment's walrus accepts only ONE sem-wait per instruction,
    while Tile emits instructions carrying several. Rewrite the BIR JSON just
    before compilation: hoist excess waits onto single-wait NoOps inserted
    immediately before the offending instruction on the same engine."""
    import json

    from concourse import bass_utils, bass2jax

    if getattr(bass_utils, "_multiwait_split_patched", False):
        return

    orig = bass_utils.compile_bir_kernel

    def _split(bir_json):
        if isinstance(bir_json, bytes):
            m = json.loads(bir_json.decode())
        else:
            m = json.loads(bir_json)
        cnt = 0
        for fn in m["functions"]:
            for bb in fn["blocks"]:
                insts = bb["instructions"]
                out = []
                for inst in insts:
                    si = inst.get("sync_info")
                    waits = si.get("on_wait", []) if si else []
                    if len(waits) > 1:
                        for w in waits[:-1]:
                            cnt += 1
                            out.append(
                                {
                                    "name": f"WS-{cnt}-{inst['name']}",
                                    "opcode": "NoOp",
                                    "engine": inst["engine"],
                                    "ins": [],
                                    "outs": [],
                                    "debug": inst.get("debug", 0),
                                    "sync_info": {
                                        "on_update": [],
                                        "on_wait": [w],
                                    },
                                }
                            )
                        si["on_wait"] = [waits[-1]]
                    out.append(inst)
                bb["instructions"] = out
        return json.dumps(m).encode()

    def patched(bir_json, tmpdir, neff_name="file.neff", **kw):
        return orig(_split(bir_json), tmpdir, neff_name=neff_name, **kw)

    bass_utils.compile_bir_kernel = patched
    bass2jax.compile_bir_kernel = patched
    bass_utils._multiwait_split_patched = True


def _build_nc():
    import contextlib

    import concourse.bass as bass
    import concourse.tile as tile
    from concourse import mybir
    from concourse.masks import make_identity

    _patch_multiwait_split()

    f32 = mybir.dt.float32
    bf16 = mybir.dt.bfloat16
    Alu = mybir.AluOpType
    Act = mybir.ActivationFunctionType

    nc = bass.Bass()
    q_d = nc.dram_tensor("q", [L, DD], f32, kind="ExternalInput")
    k_d = nc.dram_tensor("k", [KK, DD], f32, kind="ExternalInput")
    vq_d = nc.dram_tensor("vq", [L, MM * NN], f32, kind="ExternalInput")
    vk_d = nc.dram_tensor("vk", [KK, PP * NN], f32, kind="ExternalInput")
    vexp_d = nc.dram_tensor("vexp", [KK, PP], f32, kind="ExternalInput")
    scale_d = nc.dram_tensor("scale", [1, 1], f32, kind="ExternalInput")
    gamma_d = nc.dram_tensor("ln_gamma", [1, DD], f32, kind="ExternalInput")
    beta_d = nc.dram_tensor("ln_beta", [1, DD], f32, kind="ExternalInput")
    out_d = nc.dram_tensor("out", [L, MM], f32, kind="ExternalOutput")

    # (p,t)-interleaved DRAM views: partition p, slot t -> logical row 4p+t
    vk_v = vk_d[:].rearrange("(p t) c -> p t c", t=KT)
    vq_v = vq_d[:].rearrange("(p t) c -> p t c", t=LT)

    with tile.TileContext(nc) as tc:
        lp_cm = nc.allow_low_precision("bf16 value-path partial sums")
        with lp_cm, contextlib.ExitStack() as ctx:
            const = ctx.enter_context(tc.tile_pool(name="const", bufs=1))
            vk_pool = ctx.enter_context(tc.tile_pool(name="vk", bufs=2 * KT))
            vq_pool = ctx.enter_context(tc.tile_pool(name="vq", bufs=LT))
            ve_pool = ctx.enter_context(tc.tile_pool(name="ve", bufs=3))
            prod_pool = ctx.enter_context(tc.tile_pool(name="prod", bufs=3))
            tree_pool = ctx.enter_context(tc.tile_pool(name="tree", bufs=2))
            small = ctx.enter_context(tc.tile_pool(name="small", bufs=2))
            ps_scores = ctx.enter_context(
                tc.tile_pool(name="ps_s", bufs=2, space="PSUM")
            )
            ps_tmp_pool = ctx.enter_context(
                tc.tile_pool(name="ps_tmp", bufs=1, space="PSUM")
            )
            ps_tr = ctx.enter_context(tc.tile_pool(name="ps_tr", bufs=1, space="PSUM"))

            # ---- small line-rate loads on the sync HWDGE queue ----
            scale_sb = const.tile([1, 1], f32)
            nc.sync.dma_start(out=scale_sb, in_=scale_d[:])
            q_nat = const.tile([128, LT, DD], f32)
            nc.sync.dma_start(
                out=q_nat[:].rearrange("p a b -> p (a b)"),
                in_=q_d[:].rearrange("(p t) d -> p (t d)", t=LT),
            )
            k_nat = const.tile([128, KT, DD], f32)
            nc.sync.dma_start(
                out=k_nat[:].rearrange("p a b -> p (a b)"),
                in_=k_d[:].rearrange("(p t) d -> p (t d)", t=KT),
            )
            # slow splat DMAs isolated on the scalar queue (needed late)
            gamma_bc = const.tile([128, DD], f32)
            nc.scalar.dma_start(out=gamma_bc, in_=gamma_d[:].to_broadcast([128, DD]))
            beta_bc = const.tile([128, DD], f32)
            nc.scalar.dma_start(out=beta_bc, in_=beta_d[:].to_broadcast([128, DD]))

            # ---- bulk: ONE SWDGE queue, bf16 cast in flight ----
            # vexp leads (it gates the VE copies and so the whole A phase)
            vexp_nat = const.tile([128, KT, PP], f32)
            nc.gpsimd.dma_start(
                out=vexp_nat[:].rearrange("p a b -> p (a b)"),
                in_=vexp_d[:].rearrange("(p t) d -> p (t d)", t=KT),
            )
            vk_halves = []
            for h in range(KT * 2):
                i, hh = divmod(h, 2)
                vkh = vk_pool.tile([128, HALF, NN], bf16, tag="vk", name=f"vkh{h}")
                nc.gpsimd.dma_start(
                    out=vkh,
                    in_=vk_v[:, i, hh * HALF * NN : (hh + 1) * HALF * NN],
                )
                vk_halves.append(vkh)
            vq_tiles = []
            for j in range(LT):
                vqt = vq_pool.tile([128, MM, NN], bf16, tag="vq", name=f"vqt{j}")
                nc.gpsimd.dma_start(out=vqt, in_=vq_v[:, j, :])
                vq_tiles.append(vqt)

            # ---- front-end chain, force-scheduled early: identity,
            # scale broadcast, tile-0 VE copies, transposes, scores, exps.
            # Without this the compile-time scheduler lets the 16us of VE
            # copies clog ACT and the exps (and so the tmp matmuls and the
            # whole C phase) slip by ~25us.
            exp_warm = const.tile([128, 1], f32)
            identity = const.tile([128, 128], f32)
            zero_t = const.tile([128, 1], f32)
            eps_t = const.tile([128, 1], f32)
            ones_row = const.tile([1, 128], f32)
            vkc = const.tile([128, KT, NN + 1], f32)
            qT = const.tile([64, L], f32)
            kT = const.tile([64, KK], f32)
            ET = const.tile([128, KT, L], f32)
            scale_bc = const.tile([128, 1], f32)
            ve_tiles = {}
            with tc.high_priority():
                make_identity(nc, identity)
                nc.vector.memset(zero_t, 0.0)
                nc.vector.memset(eps_t, 1e-3)
                nc.vector.memset(ones_row, 1.0)
                # vkc carries a fused ones column (65th) so one matmul per
                # (k-tile, l-chunk) produces both tmp and the denominator
                nc.vector.memset(vkc[:, :, NN : NN + 1], 1.0)

                # trigger the Exp act-table load immediately so the first
                # real exp does not eat the 1.28us table latency
                nc.scalar.activation(
                    exp_warm, eps_t, func=Act.Exp, bias=0.0, scale=1.0
                )
                ps_bc = ps_tr.tile([128, 1], f32, tag="bc")
                nc.tensor.matmul(
                    ps_bc, lhsT=ones_row, rhs=scale_sb, start=True, stop=True
                )
                nc.scalar.copy(scale_bc, ps_bc)

                # tile-0 VE copies first so the A phase can start while the
                # exps are still waiting on the scores
                for hh in range(2):
                    ve0 = ve_pool.tile(
                        [128, HALF, NN], bf16, tag="ve", name=f"ve0{hh}"
                    )
                    nc.scalar.activation(
                        ve0[:],
                        vexp_nat[
                            :, 0, hh * HALF : (hh + 1) * HALF, None
                        ].to_broadcast([128, HALF, NN]),
                        func=Act.Copy,
                        bias=0.0,
                        scale=1.0,
                    )
                    ve_tiles[(0, hh)] = ve0


            # preload the Sqrt act table now (off the critical path); the
            # LN tail would otherwise eat a 1.3us table swap
            dummy = const.tile([128, 1], f32)
            nc.scalar.activation(dummy, eps_t, func=Act.Sqrt, bias=0.0, scale=1.0)

            # ---- A step + tmp/sums matmuls per k-tile ----
            ps_tmps = [
                ps_tmp_pool.tile(
                    [128, NN + 1], f32, tag=f"tmp{j}", name=f"ps_tmp{j}"
                )
                for j in range(LT)
            ]

            def dve_tree_p(pr, out_slot, w0):
                # reduce [128, w0, NN] over the middle axis down to out_slot
                cur = pr
                w = w0 // 2
                while w >= 1:
                    if w == 1:
                        nxt = out_slot
                    else:
                        nxt = tree_pool.tile(
                            [128, w, NN], bf16, tag=f"at{w}", name=f"at{w}"
                        )
                    nc.vector.tensor_tensor(
                        nxt[:], cur[:, 0:w, :], cur[:, w : 2 * w, :], Alu.add
                    )
                    cur = nxt
                    w //= 2

            def emit_transposes(which):
                # slotted into the A-loop DMA-arrival gaps so the in-order
                # DVE stream never blocks the A products on the q/k loads
                src_t, dst_t = (q_nat, qT) if which == "q" else (k_nat, kT)
                n_t = LT if which == "q" else KT
                for i in range(n_t):
                    pt = ps_tr.tile([64, 128], f32, tag="tr")
                    nc.tensor.transpose(pt, src_t[:, i, :], identity)
                    nc.vector.tensor_copy(dst_t[:, i * 128 : (i + 1) * 128], pt)

            def emit_scores():
                for i in range(KT):
                    ps_s = ps_scores.tile([128, L], f32, tag="sc")
                    nc.tensor.matmul(
                        ps_s,
                        lhsT=kT[:, i * 128 : (i + 1) * 128],
                        rhs=qT[:],
                        start=True,
                        stop=True,
                    )
                    nc.scalar.activation(
                        ET[:, i, :],
                        ps_s,
                        func=Act.Exp,
                        bias=zero_t[:],
                        scale=scale_bc[:],
                    )

            for i in range(KT):
                pr = prod_pool.tile([128, PP, NN], bf16, tag="pr", name=f"pr{i}")
                for hh in range(2):
                    # materialize the vexp broadcast as a real bf16 tile on
                    # ACT so the DVE product gets 2x (packed operands only)
                    ve = ve_tiles.get((i, hh))
                    if ve is None:
                        ve = ve_pool.tile(
                            [128, HALF, NN], bf16, tag="ve", name=f"ve{i}{hh}"
                        )
                        nc.scalar.activation(
                            ve[:],
                            vexp_nat[
                                :, i, hh * HALF : (hh + 1) * HALF, None
                            ].to_broadcast([128, HALF, NN]),
                            func=Act.Copy,
                            bias=0.0,
                            scale=1.0,
                        )
                    nc.vector.tensor_tensor(
                        pr[:, hh * HALF : (hh + 1) * HALF, :],
                        vk_halves[2 * i + hh][:],
                        ve[:],
                        Alu.mult,
                    )
                    if i == 0 and hh == 0:
                        emit_transposes("q")
                    elif i == 0 and hh == 1:
                        emit_transposes("k")
                        emit_scores()
                dve_tree_p(pr, vkc[:, i : i + 1, 0:NN], PP)
                for j in range(LT):
                    nc.tensor.matmul(
                        ps_tmps[j],
                        lhsT=ET[:, i, j * 128 : (j + 1) * 128],
                        rhs=vkc[:, i, :],
                        start=(i == 0),
                        stop=(i == KT - 1),
                    )

            # ---- fold 1/sums into tmp during the PSUM->SBUF copy ----
            tmp_sb = const.tile([128, LT, NN], bf16)
            for j in range(LT):
                recip = small.tile([128, 1], f32, tag="recip")
                nc.vector.reciprocal(recip, ps_tmps[j][:, NN : NN + 1])
                nc.scalar.activation(
                    tmp_sb[:, j, :],
                    ps_tmps[j][:, 0:NN],
                    func=Act.Copy,
                    bias=0.0,
                    scale=recip[:],
                )

            # ---- C step per l-tile: product+tree per m-half, then LN ----
            out_sb = const.tile([128, LT, MM], f32)
            for j in range(LT):
                attn = small.tile([128, MM], f32, tag="attn")
                pr2 = prod_pool.tile([128, MM, NN], bf16, tag="pr2", name=f"pr2{j}")
                nc.vector.tensor_tensor(
                    pr2[:],
                    vq_tiles[j][:],
                    tmp_sb[:, j, None, :].to_broadcast([128, MM, NN]),
                    Alu.mult,
                )
                cur = pr2
                w = NN // 2
                while w >= 1:
                    if w == 1:
                        nxt = attn[:, :, None]
                    else:
                        nxt = tree_pool.tile(
                            [128, MM, w], bf16, tag=f"ct{w}", name=f"ct{w}"
                        )
                    nc.vector.tensor_tensor(
                        nxt[:], cur[:, :, 0:w], cur[:, :, w : 2 * w], Alu.add
                    )
                    cur = nxt
                    w //= 2

                x = small.tile([128, MM], f32, tag="x")
                nc.vector.tensor_tensor(x, attn, q_nat[:, j, :], Alu.add)
                stats = small.tile([128, 6], f32, tag="stats")
                nc.vector.bn_stats(out=stats, in_=x[:])
                mv = small.tile([128, 2], f32, tag="mv")
                nc.vector.bn_aggr(out=mv, in_=stats[:])
                sd = small.tile([128, 1], f32, tag="sd")
                nc.scalar.activation(
                    sd, mv[:, 1:2], func=Act.Sqrt, bias=eps_t[:], scale=1.0
                )
                rstd = small.tile([128, 1], f32, tag="rstd")
                nc.vector.reciprocal(rstd, sd)
                xn = small.tile([128, MM], f32, tag="xn")
                nc.vector.tensor_scalar(
                    out=xn, in0=x, scalar1=mv[:, 0:1], scalar2=rstd,
                    op0=Alu.subtract, op1=Alu.mult,
                )
                xg = small.tile([128, MM], f32, tag="xg")
                nc.vector.tensor_tensor(xg, xn, gamma_bc, Alu.mult)
                nc.vector.tensor_tensor(out_sb[:, j, :], xg, beta_bc, Alu.add)

            # single line-rate store of the whole output
            nc.scalar.dma_start(
                out=out_d[:].rearrange("(p t) d -> p t d", t=LT), in_=out_sb
            )

    return nc


def _get_nc():
    if "nc" not in _CACHE:
        _CACHE["nc"] = _build_nc()
    return _CACHE["nc"]


def kernel(q, k, vq, vk, vexp, scale, ln_gamma, ln_beta):
    from concourse import bass_utils

    nc = _get_nc()
    q = np.ascontiguousarray(np.asarray(q, dtype=np.float32))
    k = np.ascontiguousarray(np.asarray(k, dtype=np.float32))
    vq = np.ascontiguousarray(np.asarray(vq, dtype=np.float32)).reshape(B, L, MM * NN)
    vk = np.ascontiguousarray(np.asarray(vk, dtype=np.float32)).reshape(B, KK, PP * NN)
    vexp = np.ascontiguousarray(np.asarray(vexp, dtype=np.float32))
    scale_arr = np.asarray(scale, dtype=np.float32).reshape(1, 1)
    gamma_arr = np.asarray(ln_gamma, dtype=np.float32).reshape(1, DD)
    beta_arr = np.asarray(ln_beta, dtype=np.float32).reshape(1, DD)

    in_maps = [
        {
            "q": q[c],
            "k": k[c],
            "vq": vq[c],
            "vk": vk[c],
            "vexp": vexp[c],
            "scale": scale_arr,
            "ln_gamma": gamma_arr,
            "ln_beta": beta_arr,
        }
        for c in range(NCORES)
    ]
    res = bass_utils.run_bass_kernel_spmd(nc, in_maps, core_ids=list(range(NCORES)))
    out = np.stack([res.results[c]["out"] for c in range(NCORES)], axis=0)
    return out.astype(np.float32)


# revision 18
# speedup vs baseline: 1.2096x; 1.0619x over previous
"""Trainium2 Bass kernel for nn_CrossAttention (B=8, L=K=512, M=N=P=D=64).

One batch per NeuronCore (8 cores, data-parallel, no collectives).

Math per batch b:
  scoresT[k,l] = scale * (K @ Q^T)                  # PE, contract D=64
  ET = exp(scale*scoresT)                           # ACT (no max-sub: |s|<~45)
  VE[k,p,n] = vexp[k,p] broadcast over n            # ACT copy (cast bf16)
  vkc[k,n] = sum_p vk[k,p,n]*VE[k,p,n]              # DVE bf16 2x mult + tree
  [tmp|sums][l,(n|1)] = ET_chunk^T @ [vkc|1]        # PE accumulate, l on partitions
  tmpn[l,n] = tmp[l,n]/sums[l]                      # ACT copy with per-partition scale
  attn[l,m] = sum_n vq[l,m,n]*tmpn[l,n]             # DVE bf16 2x mult + tree
  out = LN(attn + q)*gamma + beta                   # DVE stats, ACT sqrt

Layout: all row-indexed tensors use a (p,t) interleave — partition p of
tile t holds logical row 4p+t — so every DMA moves >=1KB contiguous per
partition (line rate). The interleave is self-consistent along both k
and l and the single output store undoes it.

Schedule (from NTFF traces of previous versions):
  - per-core HBM caps ~430 GB/s TOTAL across queues, so all 16MB of bulk
    (vk then vq, f32->bf16 cast in flight) rides ONE SWDGE queue whose
    FIFO gives vk strict priority; q/k/vexp ride the sync queue (done
    ~1.5us); only the slow to_broadcast splats (gamma/beta) and the
    final store use the scalar queue.
  - gpsimd runs NO compute: DVE ops overlapping gpsimd tensor ops were
    measured 4-8x slower (SBUF contention).
  - the A-step product runs at DVE 2x by materializing the vexp
    broadcast into a real bf16 tile on the otherwise-idle ACT engine
    (a stride-0 innermost operand would force 1x on the DVE).
  - scale is broadcast on-chip via a ones-row PE matmul instead of a
    128-descriptor splat DMA (it gates the exps).
"""

import numpy as np

B = 8
L = 512
KK = 512
MM = 64
NN = 64
PP = 64
DD = 64
NCORES = 8

LT = L // 128   # 4 l-tiles (slot index in the (p,t) interleave)
KT = KK // 128  # 4 k-tiles
HALF = 32       # p-half / m-half within a tile

_CACHE = {}


def _patch_multiwait_split():
    """This environment's walrus accepts only ONE sem-wait per instruction,
    while Tile emits instructions carrying several. Rewrite the BIR JSON just
    before compilation: hoist excess waits onto single-wait NoOps inserted
    immediately before the offending instruction on the same engine."""
    import json

    from concourse import bass_utils, bass2jax

    if getattr(bass_utils, "_multiwait_split_patched", False):
        return

    orig = bass_utils.compile_bir_kernel

    def _split(bir_json):
        if isinstance(bir_json, bytes):
            m = json.loads(bir_json.decode())
        else:
            m = json.loads(bir_json)
        cnt = 0
        for fn in m["functions"]:
            for bb in fn["blocks"]:
                insts = bb["instructions"]
                out = []
                for inst in insts:
                    si = inst.get("sync_info")
                    waits = si.get("on_wait", []) if si else []
                    if len(waits) > 1:
                        for w in waits[:-1]:
                            cnt += 1
                            out.append(
                                {
                                    "name": f"WS-{cnt}-{inst['name']}",
                                    "opcode": "NoOp",
                                    "engine": inst["engine"],
                                    "ins": [],
                                    "outs": [],
                                    "debug": inst.get("debug", 0),
                                    "sync_info": {
                                        "on_update": [],
                                        "on_wait": [w],
                                    },
                                }
                            )
                        si["on_wait"] = [waits[-1]]
                    out.append(inst)
                bb["instructions"] = out
        return json.dumps(m).encode()

    def patched(bir_json, tmpdir, neff_name="file.neff", **kw):
        return orig(_split(bir_json), tmpdir, neff_name=neff_name, **kw)

    bass_utils.compile_bir_kernel = patched
    bass2jax.compile_bir_kernel = patched
    bass_utils._multiwait_split_patched = True


def _build_nc():
    import contextlib

    import concourse.bass as bass
    import concourse.tile as tile
    from concourse import mybir
    from concourse.masks import make_identity

    _patch_multiwait_split()

    f32 = mybir.dt.float32
    bf16 = mybir.dt.bfloat16
    Alu = mybir.AluOpType
    Act = mybir.ActivationFunctionType

    nc = bass.Bass()
    q_d = nc.dram_tensor("q", [L, DD], f32, kind="ExternalInput")
    k_d = nc.dram_tensor("k", [KK, DD], f32, kind="ExternalInput")
    vq_d = nc.dram_tensor("vq", [L, MM * NN], f32, kind="ExternalInput")
    vk_d = nc.dram_tensor("vk", [KK, PP * NN], f32, kind="ExternalInput")
    vexp_d = nc.dram_tensor("vexp", [KK, PP], f32, kind="ExternalInput")
    scale_d = nc.dram_tensor("scale", [1, 1], f32, kind="ExternalInput")
    gamma_d = nc.dram_tensor("ln_gamma", [1, DD], f32, kind="ExternalInput")
    beta_d = nc.dram_tensor("ln_beta", [1, DD], f32, kind="ExternalInput")
    out_d = nc.dram_tensor("out", [L, MM], f32, kind="ExternalOutput")

    # (p,t)-interleaved DRAM views: partition p, slot t -> logical row 4p+t
    vk_v = vk_d[:].rearrange("(p t) c -> p t c", t=KT)
    vq_v = vq_d[:].rearrange("(p t) c -> p t c", t=LT)

    with tile.TileContext(nc) as tc:
        lp_cm = nc.allow_low_precision("bf16 value-path partial sums")
        with lp_cm, contextlib.ExitStack() as ctx:
            const = ctx.enter_context(tc.tile_pool(name="const", bufs=1))
            vk_pool = ctx.enter_context(tc.tile_pool(name="vk", bufs=2 * KT))
            vq_pool = ctx.enter_context(tc.tile_pool(name="vq", bufs=LT))
            ve_pool = ctx.enter_context(tc.tile_pool(name="ve", bufs=3))
            prod_pool = ctx.enter_context(tc.tile_pool(name="prod", bufs=3))
            tree_pool = ctx.enter_context(tc.tile_pool(name="tree", bufs=2))
            small = ctx.enter_context(tc.tile_pool(name="small", bufs=2))
            ps_scores = ctx.enter_context(
                tc.tile_pool(name="ps_s", bufs=2, space="PSUM")
            )
            ps_tmp_pool = ctx.enter_context(
                tc.tile_pool(name="ps_tmp", bufs=1, space="PSUM")
            )
            ps_tr = ctx.enter_context(tc.tile_pool(name="ps_tr", bufs=1, space="PSUM"))

            # ---- small line-rate loads on the sync HWDGE queue ----
            scale_sb = const.tile([1, 1], f32)
            nc.sync.dma_start(out=scale_sb, in_=scale_d[:])
            # slow splat DMAs isolated on the scalar queue (needed late)
            gamma_bc = const.tile([128, DD], f32)
            nc.scalar.dma_start(out=gamma_bc, in_=gamma_d[:].to_broadcast([128, DD]))
            beta_bc = const.tile([128, DD], f32)
            nc.scalar.dma_start(out=beta_bc, in_=beta_d[:].to_broadcast([128, DD]))

            # ---- bulk: ONE SWDGE queue, bf16 cast in flight ----
            # vexp leads (it gates the VE copies and so the whole A phase)
            vexp_nat = const.tile([128, KT, PP], f32)
            nc.gpsimd.dma_start(
                out=vexp_nat[:].rearrange("p a b -> p (a b)"),
                in_=vexp_d[:].rearrange("(p t) d -> p (t d)", t=KT),
            )
            # q/k also ride the flowing SWDGE queue: the sync queue needs
            # ~4us per small load, stalling the DVE on the qT/kT copies
            q_nat = const.tile([128, LT, DD], f32)
            nc.gpsimd.dma_start(
                out=q_nat[:].rearrange("p a b -> p (a b)"),
                in_=q_d[:].rearrange("(p t) d -> p (t d)", t=LT),
            )
            k_nat = const.tile([128, KT, DD], f32)
            nc.gpsimd.dma_start(
                out=k_nat[:].rearrange("p a b -> p (a b)"),
                in_=k_d[:].rearrange("(p t) d -> p (t d)", t=KT),
            )
            vk_halves = []
            for h in range(KT * 2):
                i, hh = divmod(h, 2)
                vkh = vk_pool.tile([128, HALF, NN], bf16, tag="vk", name=f"vkh{h}")
                nc.gpsimd.dma_start(
                    out=vkh,
                    in_=vk_v[:, i, hh * HALF * NN : (hh + 1) * HALF * NN],
                )
                vk_halves.append(vkh)
            vq_tiles = []
            for j in range(LT):
                vqt = vq_pool.tile([128, MM, NN], bf16, tag="vq", name=f"vqt{j}")
                nc.gpsimd.dma_start(out=vqt, in_=vq_v[:, j, :])
                vq_tiles.append(vqt)

            # ---- front-end chain, force-scheduled early: identity,
            # scale broadcast, tile-0 VE copies, transposes, scores, exps.
            # Without this the compile-time scheduler lets the 16us of VE
            # copies clog ACT and the exps (and so the tmp matmuls and the
            # whole C phase) slip by ~25us.
            exp_warm = const.tile([128, 1], f32)
            identity = const.tile([128, 128], f32)
            zero_t = const.tile([128, 1], f32)
            eps_t = const.tile([128, 1], f32)
            ones_row = const.tile([1, 128], f32)
            vkc = const.tile([128, KT, NN + 1], f32)
            qT = const.tile([64, L], f32)
            kT = const.tile([64, KK], f32)
            ET = const.tile([128, KT, L], f32)
            scale_bc = const.tile([128, 1], f32)
            ve_tiles = {}
            with tc.high_priority():
                make_identity(nc, identity)
                nc.vector.memset(zero_t, 0.0)
                nc.vector.memset(eps_t, 1e-3)
                nc.vector.memset(ones_row, 1.0)
                # vkc carries a fused ones column (65th) so one matmul per
                # (k-tile, l-chunk) produces both tmp and the denominator
                nc.vector.memset(vkc[:, :, NN : NN + 1], 1.0)

                # trigger the Exp act-table load immediately so the first
                # real exp does not eat the 1.28us table latency
                nc.scalar.activation(
                    exp_warm, eps_t, func=Act.Exp, bias=0.0, scale=1.0
                )
                ps_bc = ps_tr.tile([128, 1], f32, tag="bc")
                nc.tensor.matmul(
                    ps_bc, lhsT=ones_row, rhs=scale_sb, start=True, stop=True
                )
                nc.scalar.copy(scale_bc, ps_bc)

                # tile-0 VE copies first so the A phase can start while the
                # exps are still waiting on the scores
                for hh in range(2):
                    ve0 = ve_pool.tile(
                        [128, HALF, NN], bf16, tag="ve", name=f"ve0{hh}"
                    )
                    nc.scalar.activation(
                        ve0[:],
                        vexp_nat[
                            :, 0, hh * HALF : (hh + 1) * HALF, None
                        ].to_broadcast([128, HALF, NN]),
                        func=Act.Copy,
                        bias=0.0,
                        scale=1.0,
                    )
                    ve_tiles[(0, hh)] = ve0


            # preload the Sqrt act table now (off the critical path); the
            # LN tail would otherwise eat a 1.3us table swap
            dummy = const.tile([128, 1], f32)
            nc.scalar.activation(dummy, eps_t, func=Act.Sqrt, bias=0.0, scale=1.0)

            # ---- A step + tmp/sums matmuls per k-tile ----
            ps_tmps = [
                ps_tmp_pool.tile(
                    [128, NN + 1], f32, tag=f"tmp{j}", name=f"ps_tmp{j}"
                )
                for j in range(LT)
            ]

            def dve_tree_p(pr, out_slot, w0):
                # reduce [128, w0, NN] over the middle axis down to out_slot
                cur = pr
                w = w0 // 2
                while w >= 1:
                    if w == 1:
                        nxt = out_slot
                    else:
                        nxt = tree_pool.tile(
                            [128, w, NN], bf16, tag=f"at{w}", name=f"at{w}"
                        )
                    nc.vector.tensor_tensor(
                        nxt[:], cur[:, 0:w, :], cur[:, w : 2 * w, :], Alu.add
                    )
                    cur = nxt
                    w //= 2

            def emit_transposes(which):
                # slotted into the A-loop DMA-arrival gaps so the in-order
                # DVE stream never blocks the A products on the q/k loads
                src_t, dst_t = (q_nat, qT) if which == "q" else (k_nat, kT)
                n_t = LT if which == "q" else KT
                for i in range(n_t):
                    pt = ps_tr.tile([64, 128], f32, tag="tr")
                    nc.tensor.transpose(pt, src_t[:, i, :], identity)
                    nc.vector.tensor_copy(dst_t[:, i * 128 : (i + 1) * 128], pt)

            def emit_scores():
                for i in range(KT):
                    ps_s = ps_scores.tile([128, L], f32, tag="sc")
                    nc.tensor.matmul(
                        ps_s,
                        lhsT=kT[:, i * 128 : (i + 1) * 128],
                        rhs=qT[:],
                        start=True,
                        stop=True,
                    )
                    nc.scalar.activation(
                        ET[:, i, :],
                        ps_s,
                        func=Act.Exp,
                        bias=zero_t[:],
                        scale=scale_bc[:],
                    )

            for i in range(KT):
                pr = prod_pool.tile([128, PP, NN], bf16, tag="pr", name=f"pr{i}")
                for hh in range(2):
                    # materialize the vexp broadcast as a real bf16 tile on
                    # ACT so the DVE product gets 2x (packed operands only)
                    ve = ve_tiles.get((i, hh))
                    if ve is None:
                        ve = ve_pool.tile(
                            [128, HALF, NN], bf16, tag="ve", name=f"ve{i}{hh}"
                        )
                        nc.scalar.activation(
                            ve[:],
                            vexp_nat[
                                :, i, hh * HALF : (hh + 1) * HALF, None
                            ].to_broadcast([128, HALF, NN]),
                            func=Act.Copy,
                            bias=0.0,
                            scale=1.0,
                        )
                    nc.vector.tensor_tensor(
                        pr[:, hh * HALF : (hh + 1) * HALF, :],
                        vk_halves[2 * i + hh][:],
                        ve[:],
                        Alu.mult,
                    )
                    if i == 0 and hh == 0:
                        emit_transposes("q")
                    elif i == 0 and hh == 1:
                        emit_transposes("k")
                        emit_scores()
                dve_tree_p(pr, vkc[:, i : i + 1, 0:NN], PP)
                for j in range(LT):
                    nc.tensor.matmul(
                        ps_tmps[j],
                        lhsT=ET[:, i, j * 128 : (j + 1) * 128],
                        rhs=vkc[:, i, :],
                        start=(i == 0),
                        stop=(i == KT - 1),
                    )

            # ---- fold 1/sums into tmp during the PSUM->SBUF copy ----
            tmp_sb = const.tile([128, LT, NN], bf16)
            for j in range(LT):
                recip = small.tile([128, 1], f32, tag="recip")
                nc.vector.reciprocal(recip, ps_tmps[j][:, NN : NN + 1])
                nc.scalar.activation(
                    tmp_sb[:, j, :],
                    ps_tmps[j][:, 0:NN],
                    func=Act.Copy,
                    bias=0.0,
                    scale=recip[:],
                )

            # ---- C step per l-tile: product+tree per m-half, then LN ----
            out_sb = const.tile([128, LT, MM], f32)
            for j in range(LT):
                attn = small.tile([128, MM], f32, tag="attn")
                pr2 = prod_pool.tile([128, MM, NN], bf16, tag="pr2", name=f"pr2{j}")
                nc.vector.tensor_tensor(
                    pr2[:],
                    vq_tiles[j][:],
                    tmp_sb[:, j, None, :].to_broadcast([128, MM, NN]),
                    Alu.mult,
                )
                cur = pr2
                w = NN // 2
                while w >= 1:
                    if w == 1:
                        nxt = attn[:, :, None]
                    else:
                        nxt = tree_pool.tile(
                            [128, MM, w], bf16, tag=f"ct{w}", name=f"ct{w}"
                        )
                    nc.vector.tensor_tensor(
                        nxt[:], cur[:, :, 0:w], cur[:, :, w : 2 * w], Alu.add
                    )
                    cur = nxt
                    w //= 2

                x = small.tile([128, MM], f32, tag="x")
                nc.vector.tensor_tensor(x, attn, q_nat[:, j, :], Alu.add)
                stats = small.tile([128, 6], f32, tag="stats")
                nc.vector.bn_stats(out=stats, in_=x[:])
                mv = small.tile([128, 2], f32, tag="mv")
                nc.vector.bn_aggr(out=mv, in_=stats[:])
                sd = small.tile([128, 1], f32, tag="sd")
                nc.scalar.activation(
                    sd, mv[:, 1:2], func=Act.Sqrt, bias=eps_t[:], scale=1.0
                )
                rstd = small.tile([128, 1], f32, tag="rstd")
                nc.vector.reciprocal(rstd, sd)
                xn = small.tile([128, MM], f32, tag="xn")
                nc.vector.tensor_scalar(
                    out=xn, in0=x, scalar1=mv[:, 0:1], scalar2=rstd,
                    op0=Alu.subtract, op1=Alu.mult,
                )
                xg = small.tile([128, MM], f32, tag="xg")
                nc.vector.tensor_tensor(xg, xn, gamma_bc, Alu.mult)
                nc.vector.tensor_tensor(out_sb[:, j, :], xg, beta_bc, Alu.add)

            # single line-rate store of the whole output
            nc.scalar.dma_start(
                out=out_d[:].rearrange("(p t) d -> p t d", t=LT), in_=out_sb
            )

    return nc


def _get_nc():
    if "nc" not in _CACHE:
        _CACHE["nc"] = _build_nc()
    return _CACHE["nc"]


def kernel(q, k, vq, vk, vexp, scale, ln_gamma, ln_beta):
    from concourse import bass_utils

    nc = _get_nc()
    q = np.ascontiguousarray(np.asarray(q, dtype=np.float32))
    k = np.ascontiguousarray(np.asarray(k, dtype=np.float32))
    vq = np.ascontiguousarray(np.asarray(vq, dtype=np.float32)).reshape(B, L, MM * NN)
    vk = np.ascontiguousarray(np.asarray(vk, dtype=np.float32)).reshape(B, KK, PP * NN)
    vexp = np.ascontiguousarray(np.asarray(vexp, dtype=np.float32))
    scale_arr = np.asarray(scale, dtype=np.float32).reshape(1, 1)
    gamma_arr = np.asarray(ln_gamma, dtype=np.float32).reshape(1, DD)
    beta_arr = np.asarray(ln_beta, dtype=np.float32).reshape(1, DD)

    in_maps = [
        {
            "q": q[c],
            "k": k[c],
            "vq": vq[c],
            "vk": vk[c],
            "vexp": vexp[c],
            "scale": scale_arr,
            "ln_gamma": gamma_arr,
            "ln_beta": beta_arr,
        }
        for c in range(NCORES)
    ]
    res = bass_utils.run_bass_kernel_spmd(nc, in_maps, core_ids=list(range(NCORES)))
    out = np.stack([res.results[c]["out"] for c in range(NCORES)], axis=0)
    return out.astype(np.float32)
